# revision 27
# baseline (speedup 1.0000x reference)
"""Multi-head attention Trainium2 kernel (8-core SPMD), v2.

Problem: B=2, S=2048, EMBED=1024, HEADS=16, HEAD_DIM=64.
  v,k,q = split_heads(X) @ W{v,k,q}.T  (per-head, shared 64x64 weights)
  out   = softmax(q k^T / 8) v ; merge heads ; out @ Wo.T + bo

Sharding: core c -> batch b=c//4, query rows [qi*512, qi*512+512), qi=c%4.
Each core computes all 16 heads for its 512 query rows; K/V projections are
replicated inside each batch group so NO collectives are needed, and the
output is a disjoint row-slice gather on the host.

v2 design (vs v1 which was PE-bound at ~222us, ACT 64% busy):
  - ACT (exp) is the fundamental floor: 16 heads x 2048k x 512q = 16.7M
    exp elements/core at 1 elem/cycle/lane @1.2GHz ~= 14us/pair busy.
    Everything else is arranged to hide under it.
  - Heads of a pair processed SEQUENTIALLY (not interleaved), and the
    score matmuls (K=64 contraction) are issued as row-tiled pairs: the
    kb-even matmul runs in PE row groups 0-1 (operands at partitions
    0-63) CONCURRENTLY with the kb-odd matmul in row groups 2-3
    (operands at partitions 64-127, ~2x S throughput, measured 3.07x for
    4-way row tiling in the engine docs). This needs the active head's
    K^T/Q^T present in BOTH partition halves: cheap DVE 4x-mode SBUF
    copies (~0.6us/pair) build the duplicate halves.
  - Score PSUM tiles [128,1024] fp32 (2 banks: bank0=kb even, bank1=kb
    odd), double buffered; exp at FD=1024 (~1.15us) back-to-back on ACT.
  - PV unchanged: V_aug [128k, 65] per (kb, head) with a ones column so
    PSUM row 64 accumulates the softmax denominator.
  - Normalization straight from PSUM: reciprocal of po row 64 (cross
    partition-base DVE op), gpsimd partition_broadcast (input must sit at
    partition 0), multiply po rows 0-63 by the broadcast -> merged fp16.
  - fc_out accumulated EVERY 2 PAIRS into SBUF fp32 accumulators (2
    matmuls into a mix PSUM tile + one DVE add), so only et=6,7's
    contribution remains after the last pair -> small tail.
  - PSUM budget: scores 2x2 + po 2 + mix 2 = 8 banks.
"""

import os
import sys

sys.path.insert(0, "/opt/trn_rl_repo")

import numpy as np

import concourse.bass as bass
import concourse.mybir as mybir
import concourse.tile as tile
from concourse import bacc
from concourse.bass_utils import run_bass_kernel_spmd

B = 2
S = 2048
E = 1024
H = 16
D = 64
SQ = 512          # query rows per core
NCORES = 8
NPAIR = 8         # head pairs
KBLK = 16         # 128-row key blocks
FP = mybir.dt.float32

KDT = os.environ.get("KERNEL_DT", "fp16")  # fp16 | bf16 | f32r | fp32


def build_nc(kdt=None):
    kdt = kdt or KDT
    MD = {"fp16": mybir.dt.float16, "bf16": mybir.dt.bfloat16,
          "f32r": mybir.dt.float32r, "fp32": FP}[kdt]  # matmul operand dtype
    nc = bacc.Bacc("TRN2", target_bir_lowering=False, debug=False)

    ident = nc.dram_tensor("ident", [128, 128], MD, kind="ExternalInput").ap()
    xq_t = nc.dram_tensor("xq_t", [E, SQ], MD, kind="ExternalInput").ap()
    xk_t = nc.dram_tensor("xk_t", [E, S], MD, kind="ExternalInput").ap()
    xv_t = nc.dram_tensor("xv_t", [E, S], MD, kind="ExternalInput").ap()
    wq_bd = nc.dram_tensor("wq_bd", [128, 128], MD, kind="ExternalInput").ap()
    wk_bd = nc.dram_tensor("wk_bd", [128, 128], MD, kind="ExternalInput").ap()
    wv_bd = nc.dram_tensor("wv_bd", [128, 128], MD, kind="ExternalInput").ap()
    wo_t = nc.dram_tensor("wo_t", [E, E], MD, kind="ExternalInput").ap()
    bo = nc.dram_tensor("bo", [1, E], FP, kind="ExternalInput").ap()
    out = nc.dram_tensor("out", [SQ, E], FP, kind="ExternalOutput").ap()

    with tile.TileContext(nc) as tc:
        _body(tc, xq_t, xk_t, xv_t, wq_bd, wk_bd, wv_bd, wo_t, bo, ident,
              out, MD)
    nc.compile()
    return nc


def _body(tc, xq_t, xk_t, xv_t, wq_bd, wk_bd, wv_bd, wo_t, bo, ident,
          out, MD):
    """Software-pipelined emission. The Tile scheduler keeps per-engine
    FIFO order ~= emission order, so next-pair DMA loads and projections
    and the fc_out partial rounds are emitted INTERLEAVED into the
    attention chunk stream of the current pair; otherwise they serialize
    at pair boundaries behind the norm chain (observed: 21us ACT stalls
    + PE idle >3.4us -> HAM re-throttle)."""
    from contextlib import ExitStack
    nc = tc.nc
    Exp = mybir.ActivationFunctionType.Exp

    ctx = ExitStack()
    with ctx:
        wp = ctx.enter_context(tc.tile_pool(name="w", bufs=1))
        xkp = ctx.enter_context(tc.tile_pool(name="xk", bufs=3))
        xvp = ctx.enter_context(tc.tile_pool(name="xv", bufs=3))
        xqp = ctx.enter_context(tc.tile_pool(name="xq", bufs=3))
        ktp = ctx.enter_context(tc.tile_pool(name="kt", bufs=2))
        kdp = ctx.enter_context(tc.tile_pool(name="kd", bufs=2))  # dup halves
        vp = ctx.enter_context(tc.tile_pool(name="v", bufs=2))
        qtp = ctx.enter_context(tc.tile_pool(name="qt", bufs=2))
        qdp = ctx.enter_context(tc.tile_pool(name="qd", bufs=2))
        ptp = ctx.enter_context(tc.tile_pool(name="pt", bufs=4))
        mgp = ctx.enter_context(tc.tile_pool(name="mg", bufs=4))
        dnp = ctx.enter_context(tc.tile_pool(name="dn", bufs=4))
        acp = ctx.enter_context(tc.tile_pool(name="ac", bufs=1))
        obp = ctx.enter_context(tc.tile_pool(name="ob", bufs=4))
        ps_s = ctx.enter_context(tc.tile_pool(name="ps_s", bufs=2, space="PSUM"))
        ps_o = ctx.enter_context(tc.tile_pool(name="ps_o", bufs=2, space="PSUM"))
        ps_m = ctx.enter_context(tc.tile_pool(name="ps_m", bufs=2, space="PSUM"))

        # ---- weights / bias; order tuned so the startup critical path
        # (wq -> xq -> qproj, wk -> xk -> kproj -> first S -> first exp)
        # is front-loaded on the single DMA queue ----
        wq = wp.tile([128, 128], MD, tag="wq")
        wk = wp.tile([128, 128], MD, tag="wk")
        wv = wp.tile([128, 128], MD, tag="wv")
        nc.sync.dma_start(wq[:], wq_bd)
        nc.sync.dma_start(wk[:], wk_bd)
        nbias = wp.tile([128, 1], FP, tag="nbias")
        nc.gpsimd.memset(nbias[:], -4.0)
        ones16 = wp.tile([128, KBLK], FP, tag="ones16")
        nc.gpsimd.memset(ones16[:], 1.0)
        # dummy activation: preload the exp table set (~2.7us) during the
        # startup DMAs instead of on the first real exp
        warm = wp.tile([1, 8], FP, tag="warm")
        nc.gpsimd.memset(warm[:], 0.0)
        nc.scalar.activation(warm[:], warm[:], Exp, scale=1.0,
                             bias=nbias[0:1, 0:1])

        wo_tiles = [wp.tile([128, E], MD, tag=f"wo{et}", name=f"wo{et}")
                    for et in range(8)]
        id_t = wp.tile([128, 128], MD, tag="id_t")
        # fc_out SBUF accumulators (fp16 so the final round can fold them
        # into PSUM via an identity-stationary matmul), one per output tile
        acc = [acp.tile([128, 512], MD, tag=f"acc{i}", name=f"acc{i}")
               for i in range(8)]
        merged = {}
        st = {}  # pipelined per-pair tiles

        def emit_loads(p):
            xq = xqp.tile([128, SQ], MD, tag="xq")
            nc.sync.dma_start(xq[:], xq_t[p * 128:(p + 1) * 128, :])
            xk = xkp.tile([128, S], MD, tag="xk")
            for ch in range(4):
                nc.sync.dma_start(xk[:, ch * 512:(ch + 1) * 512],
                                  xk_t[p * 128:(p + 1) * 128,
                                       ch * 512:(ch + 1) * 512])
            xv = xvp.tile([128, S], MD, tag="xv")
            for ch in range(2):
                nc.sync.dma_start(xv[:, ch * 1024:(ch + 1) * 1024],
                                  xv_t[p * 128:(p + 1) * 128,
                                       ch * 1024:(ch + 1) * 1024])
            st["x", p] = (xk, xv, xq)

        def emit_kproj(p, half, dups=True):
            # K^T projection [128(d2), 2048(k)] + chunked dup halves so the
            # active head's K^T exists at BOTH partition halves for the
            # row-tiled S pairs. Pair 0 skips the dups (its S matmuls run
            # serially in one row group) to keep the startup DVE chain short.
            xk = st["x", p][0]
            if half == 0:
                kt = ktp.tile([128, S], MD, tag="kt")
                if dups:
                    kdA = kdp.tile([128, S], MD, tag="kdA")  # even @ 64:128
                    kdB = kdp.tile([128, S], MD, tag="kdB")  # odd @ 0:64
                else:
                    kdA = kdB = None
                st["k", p] = (kt, kdA, kdB)
            else:
                kt, kdA, kdB = st["k", p]
            for ch in (2 * half, 2 * half + 1):
                sl = slice(ch * 512, (ch + 1) * 512)
                ps = ps_m.tile([128, 512], FP, tag="mix")
                nc.tensor.matmul(ps[:], lhsT=wk[:], rhs=xk[:, sl],
                                 start=True, stop=True)
                nc.vector.tensor_copy(kt[:, sl], ps[:])
                if dups:
                    nc.vector.tensor_copy(kdA[64:128, sl], kt[0:64, sl])
                    nc.vector.tensor_copy(kdB[0:64, sl], kt[64:128, sl])

        def emit_vproj(p, half):
            # V natural projection with ones columns (col 64 of each head
            # block, so the softmax denominator lands in po row 64)
            xv = st["x", p][1]
            if half == 0:
                v = vp.tile([128, KBLK * 130], MD, tag="v")
                vr = v[:].rearrange("p (b c) -> p b c", c=130)
                nc.vector.tensor_copy(vr[:, :, 64:65], ones16[:])
                nc.vector.tensor_copy(vr[:, :, 129:130], ones16[:])
                st["v", p] = v
            else:
                v = st["v", p]
            for vg in (2 * half, 2 * half + 1):
                ps = ps_m.tile([128, 512], FP, tag="mix")
                for j in range(4):
                    kb = vg * 4 + j
                    nc.tensor.matmul(ps[:, j * 128:(j + 1) * 128],
                                     lhsT=xv[:, kb * 128:(kb + 1) * 128],
                                     rhs=wv[:], start=True, stop=True)
                src4 = ps[:].rearrange("p (b g c) -> p b g c", g=2, c=64)
                dst4 = v[:, vg * 520:(vg + 1) * 520].rearrange(
                    "p (b g c) -> p b g c", g=2, c=65)[:, :, :, 0:64]
                nc.vector.tensor_copy(dst4, src4)

        def emit_qproj(p, dups=True):
            xq = st["x", p][2]
            qt = qtp.tile([128, SQ], MD, tag="qt")
            psq = ps_m.tile([128, 512], FP, tag="mix")
            nc.tensor.matmul(psq[:], lhsT=wq[:], rhs=xq[:],
                             start=True, stop=True)
            nc.vector.tensor_copy(qt[:], psq[:])
            if dups:
                qdA = qdp.tile([128, SQ], MD, tag="qdA")
                nc.vector.tensor_copy(qdA[64:128, :], qt[0:64, :])
                qdB = qdp.tile([128, SQ], MD, tag="qdB")
                nc.vector.tensor_copy(qdB[0:64, :], qt[64:128, :])
            else:
                qdA = qdB = None
            st["q", p] = (qt, qdA, qdB)

        def emit_fc_tile(pa, pb, i):
            # one fc_out output tile: acc[i] (+)= merged[pa] @ wo[pa]
            #                                  + merged[pb] @ wo[pb]
            sb, nch = i // 2, i % 2
            psf_t = ps_m.tile([128, 512], FP, tag="mix", name="psf")
            psf = psf_t[:]
            nc.tensor.matmul(
                psf,
                lhsT=merged[pa][:, sb * 128:(sb + 1) * 128],
                rhs=wo_tiles[pa][:, nch * 512:(nch + 1) * 512],
                start=True, stop=False, skip_group_check=True)
            nc.tensor.matmul(
                psf,
                lhsT=merged[pb][:, sb * 128:(sb + 1) * 128],
                rhs=wo_tiles[pb][:, nch * 512:(nch + 1) * 512],
                start=False, stop=True, skip_group_check=True)
            if pa == 0:
                nc.vector.tensor_add(acc[i][:], psf,
                                     bo_b[:, nch * 512:(nch + 1) * 512])
            else:
                nc.vector.tensor_add(acc[i][:], acc[i][:], psf)

        # final fc round, split so the norm(7,hp1)-independent matmuls
        # (merged[6], merged[7] top half, identity*acc) keep the PE warm
        # while the last norm chain runs on DVE/GpSimd; evacuation
        # alternates ACT (idle after the last exp) and DVE.
        psfs = {}

        def fc_final_early(i):
            sb, nch = i // 2, i % 2
            if i % 2 == 0:
                big = ps_s.tile([128, 1024], FP, tag="s", name=f"fcf{i}")
                psf = big[:, 0:512]
            else:
                psf_t = ps_m.tile([128, 512], FP, tag="mix", name="psf")
                psf = psf_t[:]
            nc.tensor.matmul(
                psf, lhsT=merged[6][:, sb * 128:(sb + 1) * 128],
                rhs=wo_tiles[6][:, nch * 512:(nch + 1) * 512],
                start=True, stop=False, skip_group_check=True)
            nc.tensor.matmul(
                psf, lhsT=merged[7][0:64, sb * 128:(sb + 1) * 128],
                rhs=wo_tiles[7][0:64, nch * 512:(nch + 1) * 512],
                start=False, stop=False, skip_group_check=True)
            nc.tensor.matmul(
                psf, lhsT=id_t[:], rhs=acc[i][:],
                start=False, stop=False, skip_group_check=True)
            psfs[i] = psf

        def fc_final_late(i):
            sb, nch = i // 2, i % 2
            psf = psfs.pop(i)
            nc.tensor.matmul(
                psf, lhsT=merged[7][64:128, sb * 128:(sb + 1) * 128],
                rhs=wo_tiles[7][64:128, nch * 512:(nch + 1) * 512],
                start=False, stop=True, skip_group_check=True)
            ot = obp.tile([128, 512], FP, tag="ob")
            if i % 2 == 0:
                nc.scalar.copy(ot[:], psf)
            else:
                nc.vector.tensor_copy(ot[:], psf)
            nc.sync.dma_start(
                out[sb * 128:(sb + 1) * 128,
                    nch * 512:(nch + 1) * 512], ot[:])

        def emit_norm(p, hp, po):
            # normalize: denominator row 64 -> partition 0 via a standard
            # copy (honors AP partition offsets; custom-DVE recip and
            # gpsimd broadcast need input physically at partition 0),
            # then multiply po rows 0-63 straight from PSUM (base 0).
            mg = mgp.tile([128, SQ], MD, name=f"m{p}", tag="mg") \
                if hp == 0 else merged[p]
            merged[p] = mg
            dn = dnp.tile([1, 512], FP, tag="dn")
            nc.vector.tensor_copy(dn[0:1, :], po[64:65, :])
            dr = dnp.tile([1, 512], FP, tag="dr")
            nc.vector.reciprocal_approx_fast(dr[0:1, :], dn[0:1, :])
            db = dnp.tile([64, 512], FP, tag="db")
            nc.gpsimd.partition_broadcast(db[:], dr[0:1, :], channels=64)
            nc.vector.tensor_mul(mg[hp * 64:(hp + 1) * 64, :],
                                 po[0:64, :], db[:])

        # ---- prologue: pair 0 fully, pair 1 loads ----
        emit_loads(0)
        nc.sync.dma_start(wv[:], wv_bd)
        bo_row = wp.tile([1, E], FP, tag="bo_row")
        nc.sync.dma_start(bo_row[:], bo)
        bo_b = wp.tile([128, E], FP, tag="bo_b")
        nc.gpsimd.partition_broadcast(bo_b[:], bo_row[0:1, :], channels=128)
        emit_qproj(0, dups=False)
        emit_kproj(0, 0, dups=False)
        emit_kproj(0, 1, dups=False)
        emit_vproj(0, 0)
        emit_vproj(0, 1)
        emit_loads(1)

        # ---- attention: one flat chunk stream, software-pipelined two
        # deep (S matmuls at n, exp at n-1, PV at n-2) so the ACT queue
        # always holds a ready exp and semaphore latency is hidden across
        # chunk, head, and pair boundaries. Heads sequential; S matmuls
        # issued as row-tiled pairs (kb even in PE rows 0-63, kb odd in
        # 64-127, concurrent on HW); exp FD=1024; PV accumulates po.
        chunks = [(p, hp, c) for p in range(NPAIR) for hp in range(2)
                  for c in range(8)]
        NCH = len(chunks)
        sps, pts, pos = {}, {}, {}

        def stage_S(n):
            p, hp, c = chunks[n]
            if p == 1 and hp == 0 and c == 0:
                # fc weights aren't needed until pair 2's fc round
                for et in range(8):
                    nc.sync.dma_start(wo_tiles[et][:],
                                      wo_t[et * 128:(et + 1) * 128, :])
                nc.sync.dma_start(id_t[:], ident)
            kt, kdA, kdB = st["k", p]
            qt, qdA, qdB = st["q", p]
            if kdA is None:
                # pair 0: no dup tiles; both S matmuls in one row group
                # (serial on PE - fine, startup has PE slack)
                r0 = r1 = slice(0, 64) if hp == 0 else slice(64, 128)
                kt0 = kt1 = kt
                qt0 = qt1 = qt
            elif hp == 0:
                kt0, qt0, r0 = kt, qt, slice(0, 64)
                kt1, qt1, r1 = kdA, qdA, slice(64, 128)
            else:
                kt0, qt0, r0 = kdB, qdB, slice(0, 64)
                kt1, qt1, r1 = kt, qt, slice(64, 128)
            kb0, kb1 = 2 * c, 2 * c + 1
            ps = ps_s.tile([128, 1024], FP, tag="s", name=f"s{p}_{hp}_{c}")
            nc.tensor.matmul(
                ps[:, 0:512],
                lhsT=kt0[r0, kb0 * 128:(kb0 + 1) * 128],
                rhs=qt0[r0, :],
                start=True, stop=True)
            nc.tensor.matmul(
                ps[:, 512:1024],
                lhsT=kt1[r1, kb1 * 128:(kb1 + 1) * 128],
                rhs=qt1[r1, :],
                start=True, stop=True)
            sps[n] = ps

        def stage_exp(n):
            # exp(s/8 - 4): the -4 shift cancels in softmax and keeps
            # max P ~= e^7 well inside fp16 range
            pt_ = ptp.tile([128, 1024], MD, name="pt_")
            nc.scalar.activation(pt_[:], sps.pop(n)[:], Exp,
                                 scale=0.125, bias=nbias[:])
            pts[n] = pt_

        def stage_pv(n):
            p, hp, c = chunks[n]
            if c == 0:
                pos[p, hp] = ps_o.tile([65, 512], FP, tag="o",
                                       name=f"po{p}_{hp}")
            po = pos[p, hp]
            v = st["v", p]
            pt_ = pts.pop(n)
            kb0, kb1 = 2 * c, 2 * c + 1
            nc.tensor.matmul(
                po[:],
                lhsT=v[:, kb0 * 130 + hp * 65:kb0 * 130 + hp * 65 + 65],
                rhs=pt_[:, 0:512],
                start=(c == 0), stop=False, skip_group_check=True)
            nc.tensor.matmul(
                po[:],
                lhsT=v[:, kb1 * 130 + hp * 65:kb1 * 130 + hp * 65 + 65],
                rhs=pt_[:, 512:1024],
                start=False, stop=(c == 7), skip_group_check=True)
            if c == 7:
                emit_norm(p, hp, pos.pop((p, hp)))

        def interleave(n):
            # next-pair projections and fc-round tiles, one small piece per
            # chunk slot so no emission point exceeds the per-chunk ACT
            # budget (~1.1us)
            p, hp, c = chunks[n]
            t = hp * 8 + c
            if t == 1 and p < 6:
                emit_loads(p + 2)
            elif t == 2 and p < 7:
                emit_kproj(p + 1, 0)
            elif t == 3 and p < 7:
                emit_kproj(p + 1, 1)
            elif t == 4 and p < 7:
                emit_vproj(p + 1, 0)
            elif t == 5 and p < 7:
                emit_vproj(p + 1, 1)
            elif t == 6 and p < 7:
                emit_qproj(p + 1)
            elif 7 <= t <= 14 and p >= 2 and p % 2 == 0:
                emit_fc_tile(p - 2, p - 1, t - 7)

        for n in range(NCH + 2):
            if n < NCH:
                stage_S(n)
            if 0 <= n - 1 < NCH:
                stage_exp(n - 1)
            if 0 <= n - 2 < NCH:
                stage_pv(n - 2)
            if n < NCH:
                interleave(n)

        for i in range(4):
            fc_final_early(i)
        for i in range(4):
            fc_final_late(i)
            fc_final_early(i + 4)
        for i in range(4, 8):
            fc_final_late(i)


# ---------------------------------------------------------------------------
# host side
# ---------------------------------------------------------------------------

_NC_CACHE = {}


def _get_nc():
    if KDT not in _NC_CACHE:
        _NC_CACHE[KDT] = build_nc(KDT)
    return _NC_CACHE[KDT]


def _np_dt():
    if KDT == "bf16":
        import ml_dtypes
        return ml_dtypes.bfloat16
    if KDT == "fp16":
        return np.float16
    return np.float32


def _bd(w):
    """128x128 block-diag of W.T (two copies)."""
    wt = np.ascontiguousarray(np.asarray(w).T.astype(np.float32))
    o = np.zeros((128, 128), np.float32)
    o[:64, :64] = wt
    o[64:, 64:] = wt
    return o


def kernel(values, keys, queries, Wv, Wk, Wq, Wo, bo):
    values = np.asarray(values, np.float32)
    keys = np.asarray(keys, np.float32)
    queries = np.asarray(queries, np.float32)

    dt = _np_dt()
    ident = np.eye(128, dtype=np.float32).astype(dt)
    wq_bd = _bd(Wq).astype(dt)
    wk_bd = _bd(Wk).astype(dt)
    wv_bd = _bd(Wv).astype(dt)
    wo_t = np.ascontiguousarray(np.asarray(Wo, np.float32).T).astype(dt)
    bo_r = np.ascontiguousarray(np.asarray(bo, np.float32).reshape(1, E))

    xk_t = [np.ascontiguousarray(keys[b].T).astype(dt) for b in range(B)]
    xv_t = [np.ascontiguousarray(values[b].T).astype(dt) for b in range(B)]

    in_maps = []
    for c in range(NCORES):
        b, qi = c // 4, c % 4
        in_maps.append({
            "xq_t": np.ascontiguousarray(
                queries[b, qi * SQ:(qi + 1) * SQ, :].T).astype(dt),
            "xk_t": xk_t[b],
            "xv_t": xv_t[b],
            "wq_bd": wq_bd, "wk_bd": wk_bd, "wv_bd": wv_bd,
            "wo_t": wo_t, "bo": bo_r, "ident": ident,
        })

    nc = _get_nc()
    res = run_bass_kernel_spmd(nc, in_maps, list(range(NCORES)),
                               trace=bool(int(os.environ.get("BASS_TRACE", "0"))))
    full = np.empty((B, S, E), np.float32)
    for c in range(NCORES):
        b, qi = c // 4, c % 4
        full[b, qi * SQ:(qi + 1) * SQ, :] = res.results[c]["out"]
    kernel.last_results = res
    return full


# revision 28
# speedup vs baseline: 1.1229x; 1.1229x over previous
"""Multi-head attention Trainium2 kernel (8-core SPMD), v2.

Problem: B=2, S=2048, EMBED=1024, HEADS=16, HEAD_DIM=64.
  v,k,q = split_heads(X) @ W{v,k,q}.T  (per-head, shared 64x64 weights)
  out   = softmax(q k^T / 8) v ; merge heads ; out @ Wo.T + bo

Sharding: core c -> batch b=c//4, query rows [qi*512, qi*512+512), qi=c%4.
Each core computes all 16 heads for its 512 query rows; K/V projections are
replicated inside each batch group so NO collectives are needed, and the
output is a disjoint row-slice gather on the host.

v2 design (vs v1 which was PE-bound at ~222us, ACT 64% busy):
  - ACT (exp) is the fundamental floor: 16 heads x 2048k x 512q = 16.7M
    exp elements/core at 1 elem/cycle/lane @1.2GHz ~= 14us/pair busy.
    Everything else is arranged to hide under it.
  - Heads of a pair processed SEQUENTIALLY (not interleaved), and the
    score matmuls (K=64 contraction) are issued as row-tiled pairs: the
    kb-even matmul runs in PE row groups 0-1 (operands at partitions
    0-63) CONCURRENTLY with the kb-odd matmul in row groups 2-3
    (operands at partitions 64-127, ~2x S throughput, measured 3.07x for
    4-way row tiling in the engine docs). This needs the active head's
    K^T/Q^T present in BOTH partition halves: cheap DVE 4x-mode SBUF
    copies (~0.6us/pair) build the duplicate halves.
  - Score PSUM tiles [128,1024] fp32 (2 banks: bank0=kb even, bank1=kb
    odd), double buffered; exp at FD=1024 (~1.15us) back-to-back on ACT.
  - PV unchanged: V_aug [128k, 65] per (kb, head) with a ones column so
    PSUM row 64 accumulates the softmax denominator.
  - Normalization straight from PSUM: reciprocal of po row 64 (cross
    partition-base DVE op), gpsimd partition_broadcast (input must sit at
    partition 0), multiply po rows 0-63 by the broadcast -> merged fp16.
  - fc_out accumulated EVERY 2 PAIRS into SBUF fp32 accumulators (2
    matmuls into a mix PSUM tile + one DVE add), so only et=6,7's
    contribution remains after the last pair -> small tail.
  - PSUM budget: scores 2x2 + po 2 + mix 2 = 8 banks.
"""

import os
import sys

sys.path.insert(0, "/opt/trn_rl_repo")

import numpy as np

import concourse.bass as bass
import concourse.mybir as mybir
import concourse.tile as tile
from concourse import bacc
from concourse.bass_utils import run_bass_kernel_spmd

B = 2
S = 2048
E = 1024
H = 16
D = 64
SQ = 512          # query rows per core
NCORES = 8
NPAIR = 8         # head pairs
KBLK = 16         # 128-row key blocks
FP = mybir.dt.float32

KDT = os.environ.get("KERNEL_DT", "fp16")  # fp16 | bf16 | f32r | fp32


def build_nc(kdt=None):
    kdt = kdt or KDT
    MD = {"fp16": mybir.dt.float16, "bf16": mybir.dt.bfloat16,
          "f32r": mybir.dt.float32r, "fp32": FP}[kdt]  # matmul operand dtype
    nc = bacc.Bacc("TRN2", target_bir_lowering=False, debug=False)

    ident = nc.dram_tensor("ident", [128, 128], MD, kind="ExternalInput").ap()
    xq_t = nc.dram_tensor("xq_t", [E, SQ], MD, kind="ExternalInput").ap()
    xk_t = nc.dram_tensor("xk_t", [E, S], MD, kind="ExternalInput").ap()
    xv_t = nc.dram_tensor("xv_t", [E, S], MD, kind="ExternalInput").ap()
    wq_bd = nc.dram_tensor("wq_bd", [128, 128], MD, kind="ExternalInput").ap()
    wk_bd = nc.dram_tensor("wk_bd", [128, 128], MD, kind="ExternalInput").ap()
    wv_bd = nc.dram_tensor("wv_bd", [128, 128], MD, kind="ExternalInput").ap()
    wo_t = nc.dram_tensor("wo_t", [E, E], MD, kind="ExternalInput").ap()
    bo = nc.dram_tensor("bo", [1, E], FP, kind="ExternalInput").ap()
    out = nc.dram_tensor("out", [SQ, E], FP, kind="ExternalOutput").ap()

    with tile.TileContext(nc) as tc:
        _body(tc, xq_t, xk_t, xv_t, wq_bd, wk_bd, wv_bd, wo_t, bo, ident,
              out, MD)
    nc.compile()
    return nc


def _body(tc, xq_t, xk_t, xv_t, wq_bd, wk_bd, wv_bd, wo_t, bo, ident,
          out, MD):
    """Software-pipelined emission. The Tile scheduler keeps per-engine
    FIFO order ~= emission order, so next-pair DMA loads and projections
    and the fc_out partial rounds are emitted INTERLEAVED into the
    attention chunk stream of the current pair; otherwise they serialize
    at pair boundaries behind the norm chain (observed: 21us ACT stalls
    + PE idle >3.4us -> HAM re-throttle)."""
    from contextlib import ExitStack
    nc = tc.nc
    Exp = mybir.ActivationFunctionType.Exp

    ctx = ExitStack()
    with ctx:
        wp = ctx.enter_context(tc.tile_pool(name="w", bufs=1))
        xkp = ctx.enter_context(tc.tile_pool(name="xk", bufs=3))
        xvp = ctx.enter_context(tc.tile_pool(name="xv", bufs=3))
        xqp = ctx.enter_context(tc.tile_pool(name="xq", bufs=3))
        ktp = ctx.enter_context(tc.tile_pool(name="kt", bufs=2))
        kdp = ctx.enter_context(tc.tile_pool(name="kd", bufs=2))  # dup halves
        vp = ctx.enter_context(tc.tile_pool(name="v", bufs=2))
        qtp = ctx.enter_context(tc.tile_pool(name="qt", bufs=2))
        qdp = ctx.enter_context(tc.tile_pool(name="qd", bufs=2))
        ptp = ctx.enter_context(tc.tile_pool(name="pt", bufs=4))
        mgp = ctx.enter_context(tc.tile_pool(name="mg", bufs=4))
        dnp = ctx.enter_context(tc.tile_pool(name="dn", bufs=4))
        acp = ctx.enter_context(tc.tile_pool(name="ac", bufs=1))
        obp = ctx.enter_context(tc.tile_pool(name="ob", bufs=4))
        ps_s = ctx.enter_context(tc.tile_pool(name="ps_s", bufs=2, space="PSUM"))
        ps_o = ctx.enter_context(tc.tile_pool(name="ps_o", bufs=2, space="PSUM"))
        ps_m = ctx.enter_context(tc.tile_pool(name="ps_m", bufs=2, space="PSUM"))

        # ---- weights / bias; order tuned so the startup critical path
        # (wq -> xq -> qproj, wk -> xk -> kproj -> first S -> first exp)
        # is front-loaded on the single DMA queue ----
        wq = wp.tile([128, 128], MD, tag="wq")
        wk = wp.tile([128, 128], MD, tag="wk")
        wv = wp.tile([128, 128], MD, tag="wv")
        nc.sync.dma_start(wq[:], wq_bd)
        nc.sync.dma_start(wk[:], wk_bd)
        nbias = wp.tile([128, 1], FP, tag="nbias")
        nc.gpsimd.memset(nbias[:], -4.0)
        ones16 = wp.tile([128, KBLK], FP, tag="ones16")
        nc.gpsimd.memset(ones16[:], 1.0)
        # dummy activation: preload the exp table set (~2.7us) during the
        # startup DMAs instead of on the first real exp
        warm = wp.tile([1, 8], FP, tag="warm")
        nc.gpsimd.memset(warm[:], 0.0)
        nc.scalar.activation(warm[:], warm[:], Exp, scale=1.0,
                             bias=nbias[0:1, 0:1])

        wo_tiles = [wp.tile([128, E], MD, tag=f"wo{et}", name=f"wo{et}")
                    for et in range(8)]
        id_t = wp.tile([128, 128], MD, tag="id_t")
        # fc_out SBUF accumulators (fp16 so the final round can fold them
        # into PSUM via an identity-stationary matmul), one per output tile
        acc = [acp.tile([128, 512], MD, tag=f"acc{i}", name=f"acc{i}")
               for i in range(8)]
        merged = {}
        st = {}  # pipelined per-pair tiles

        def emit_loads(p):
            xq = xqp.tile([128, SQ], MD, tag="xq")
            nc.sync.dma_start(xq[:], xq_t[p * 128:(p + 1) * 128, :])
            xk = xkp.tile([128, S], MD, tag="xk")
            for ch in range(4):
                nc.sync.dma_start(xk[:, ch * 512:(ch + 1) * 512],
                                  xk_t[p * 128:(p + 1) * 128,
                                       ch * 512:(ch + 1) * 512])
            xv = xvp.tile([128, S], MD, tag="xv")
            for ch in range(2):
                nc.sync.dma_start(xv[:, ch * 1024:(ch + 1) * 1024],
                                  xv_t[p * 128:(p + 1) * 128,
                                       ch * 1024:(ch + 1) * 1024])
            st["x", p] = (xk, xv, xq)

        def emit_kproj(p, half, dups=True):
            # K^T projection [128(d2), 2048(k)] + chunked dup halves so the
            # active head's K^T exists at BOTH partition halves for the
            # row-tiled S pairs. Pair 0 skips the dups (its S matmuls run
            # serially in one row group) to keep the startup DVE chain short.
            xk = st["x", p][0]
            if half == 0:
                kt = ktp.tile([128, S], MD, tag="kt")
                if dups:
                    kdA = kdp.tile([128, S], MD, tag="kdA")  # even @ 64:128
                    kdB = kdp.tile([128, S], MD, tag="kdB")  # odd @ 0:64
                else:
                    kdA = kdB = None
                st["k", p] = (kt, kdA, kdB)
            else:
                kt, kdA, kdB = st["k", p]
            for ch in (2 * half, 2 * half + 1):
                sl = slice(ch * 512, (ch + 1) * 512)
                ps = ps_m.tile([128, 512], FP, tag="mix")
                nc.tensor.matmul(ps[:], lhsT=wk[:], rhs=xk[:, sl],
                                 start=True, stop=True)
                nc.vector.tensor_copy(kt[:, sl], ps[:])
                if dups:
                    nc.vector.tensor_copy(kdA[64:128, sl], kt[0:64, sl])
                    nc.vector.tensor_copy(kdB[0:64, sl], kt[64:128, sl])

        def emit_vproj(p, half):
            # V natural projection with ones columns (col 64 of each head
            # block, so the softmax denominator lands in po row 64)
            xv = st["x", p][1]
            if half == 0:
                v = vp.tile([128, KBLK * 130], MD, tag="v")
                vr = v[:].rearrange("p (b c) -> p b c", c=130)
                nc.vector.tensor_copy(vr[:, :, 64:65], ones16[:])
                nc.vector.tensor_copy(vr[:, :, 129:130], ones16[:])
                st["v", p] = v
            else:
                v = st["v", p]
            for vg in (2 * half, 2 * half + 1):
                ps = ps_m.tile([128, 512], FP, tag="mix")
                for j in range(4):
                    kb = vg * 4 + j
                    nc.tensor.matmul(ps[:, j * 128:(j + 1) * 128],
                                     lhsT=xv[:, kb * 128:(kb + 1) * 128],
                                     rhs=wv[:], start=True, stop=True)
                src4 = ps[:].rearrange("p (b g c) -> p b g c", g=2, c=64)
                dst4 = v[:, vg * 520:(vg + 1) * 520].rearrange(
                    "p (b g c) -> p b g c", g=2, c=65)[:, :, :, 0:64]
                nc.vector.tensor_copy(dst4, src4)

        def emit_qproj(p, dups=True):
            xq = st["x", p][2]
            qt = qtp.tile([128, SQ], MD, tag="qt")
            psq = ps_m.tile([128, 512], FP, tag="mix")
            nc.tensor.matmul(psq[:], lhsT=wq[:], rhs=xq[:],
                             start=True, stop=True)
            nc.vector.tensor_copy(qt[:], psq[:])
            if dups:
                qdA = qdp.tile([128, SQ], MD, tag="qdA")
                nc.vector.tensor_copy(qdA[64:128, :], qt[0:64, :])
                qdB = qdp.tile([128, SQ], MD, tag="qdB")
                nc.vector.tensor_copy(qdB[0:64, :], qt[64:128, :])
            else:
                qdA = qdB = None
            st["q", p] = (qt, qdA, qdB)

        def emit_fc_tile(pa, pb, i):
            # one fc_out output tile: acc[i] (+)= merged[pa] @ wo[pa]
            #                                  + merged[pb] @ wo[pb]
            sb, nch = i // 2, i % 2
            psf_t = ps_m.tile([128, 512], FP, tag="mix", name="psf")
            psf = psf_t[:]
            nc.tensor.matmul(
                psf,
                lhsT=merged[pa][:, sb * 128:(sb + 1) * 128],
                rhs=wo_tiles[pa][:, nch * 512:(nch + 1) * 512],
                start=True, stop=False, skip_group_check=True)
            nc.tensor.matmul(
                psf,
                lhsT=merged[pb][:, sb * 128:(sb + 1) * 128],
                rhs=wo_tiles[pb][:, nch * 512:(nch + 1) * 512],
                start=False, stop=True, skip_group_check=True)
            if pa == 0:
                nc.vector.tensor_add(acc[i][:], psf,
                                     bo_b[:, nch * 512:(nch + 1) * 512])
            else:
                nc.vector.tensor_add(acc[i][:], acc[i][:], psf)

        # final fc round, split so the norm(7,hp1)-independent matmuls
        # (merged[6], merged[7] top half, identity*acc) keep the PE warm
        # while the last norm chain runs on DVE/GpSimd; evacuation
        # alternates ACT (idle after the last exp) and DVE.
        psfs = {}

        def fc_final_early(i):
            sb, nch = i // 2, i % 2
            if i % 2 == 0:
                big = ps_s.tile([128, 1024], FP, tag="s", name=f"fcf{i}")
                psf = big[:, 0:512]
            else:
                psf_t = ps_m.tile([128, 512], FP, tag="mix", name="psf")
                psf = psf_t[:]
            nc.tensor.matmul(
                psf, lhsT=merged[6][:, sb * 128:(sb + 1) * 128],
                rhs=wo_tiles[6][:, nch * 512:(nch + 1) * 512],
                start=True, stop=False, skip_group_check=True)
            nc.tensor.matmul(
                psf, lhsT=merged[7][0:64, sb * 128:(sb + 1) * 128],
                rhs=wo_tiles[7][0:64, nch * 512:(nch + 1) * 512],
                start=False, stop=False, skip_group_check=True)
            nc.tensor.matmul(
                psf, lhsT=id_t[:], rhs=acc[i][:],
                start=False, stop=False, skip_group_check=True)
            psfs[i] = psf

        def fc_final_late(i):
            sb, nch = i // 2, i % 2
            psf = psfs.pop(i)
            nc.tensor.matmul(
                psf, lhsT=merged[7][64:128, sb * 128:(sb + 1) * 128],
                rhs=wo_tiles[7][64:128, nch * 512:(nch + 1) * 512],
                start=False, stop=True, skip_group_check=True)
            ot = obp.tile([128, 512], FP, tag="ob")
            if i % 2 == 0:
                nc.scalar.copy(ot[:], psf)
            else:
                nc.vector.tensor_copy(ot[:], psf)
            nc.sync.dma_start(
                out[sb * 128:(sb + 1) * 128,
                    nch * 512:(nch + 1) * 512], ot[:])

        def emit_norm(p, hp, po):
            # normalize: denominator row 64 -> partition 0 via a standard
            # copy (honors AP partition offsets; custom-DVE recip and
            # gpsimd broadcast need input physically at partition 0),
            # then multiply po rows 0-63 straight from PSUM (base 0).
            mg = mgp.tile([128, SQ], MD, name=f"m{p}", tag="mg") \
                if hp == 0 else merged[p]
            merged[p] = mg
            dn = dnp.tile([1, 512], FP, tag="dn")
            nc.vector.tensor_copy(dn[0:1, :], po[64:65, :])
            dr = dnp.tile([1, 512], FP, tag="dr")
            nc.vector.reciprocal_approx_fast(dr[0:1, :], dn[0:1, :])
            db = dnp.tile([64, 512], FP, tag="db")
            nc.gpsimd.partition_broadcast(db[:], dr[0:1, :], channels=64)
            nc.vector.tensor_mul(mg[hp * 64:(hp + 1) * 64, :],
                                 po[0:64, :], db[:])

        # ---- prologue: pair 0 fully, pair 1 loads ----
        emit_loads(0)
        nc.sync.dma_start(wv[:], wv_bd)
        bo_row = wp.tile([1, E], FP, tag="bo_row")
        nc.sync.dma_start(bo_row[:], bo)
        bo_b = wp.tile([128, E], FP, tag="bo_b")
        nc.gpsimd.partition_broadcast(bo_b[:], bo_row[0:1, :], channels=128)
        emit_qproj(0, dups=False)
        emit_kproj(0, 0, dups=False)
        emit_kproj(0, 1, dups=False)
        emit_vproj(0, 0)
        emit_vproj(0, 1)
        emit_loads(1)

        # ---- attention: one flat chunk stream, software-pipelined ONE
        # deep (S+exp at n, PV at n-1) - the next chunk's S pair enters the
        # PE FIFO ahead of the previous chunk's PV, keeping ACT fed across
        # chunk, head, and pair boundaries. (A two-deep variant measured
        # WORSE: exp durations grew 1113->1335ns from the extra concurrent
        # PSUM traffic.) Heads sequential; S matmuls issued as row-tiled
        # pairs (kb even in PE rows 0-63, kb odd in 64-127, concurrent on
        # HW); exp FD=1024; PV accumulates po.
        chunks = [(p, hp, c) for p in range(NPAIR) for hp in range(2)
                  for c in range(8)]
        NCH = len(chunks)
        pts, pos = {}, {}

        def stage_S_exp(n):
            p, hp, c = chunks[n]
            if p == 1 and hp == 0 and c == 0:
                # fc weights aren't needed until pair 2's fc round
                for et in range(8):
                    nc.sync.dma_start(wo_tiles[et][:],
                                      wo_t[et * 128:(et + 1) * 128, :])
                nc.sync.dma_start(id_t[:], ident)
            kt, kdA, kdB = st["k", p]
            qt, qdA, qdB = st["q", p]
            if kdA is None:
                # pair 0: no dup tiles; both S matmuls in one row group
                # (serial on PE - fine, startup has PE slack)
                r0 = r1 = slice(0, 64) if hp == 0 else slice(64, 128)
                kt0 = kt1 = kt
                qt0 = qt1 = qt
            elif hp == 0:
                kt0, qt0, r0 = kt, qt, slice(0, 64)
                kt1, qt1, r1 = kdA, qdA, slice(64, 128)
            else:
                kt0, qt0, r0 = kdB, qdB, slice(0, 64)
                kt1, qt1, r1 = kt, qt, slice(64, 128)
            kb0, kb1 = 2 * c, 2 * c + 1
            ps = ps_s.tile([128, 1024], FP, tag="s", name=f"s{p}_{hp}_{c}")
            nc.tensor.matmul(
                ps[:, 0:512],
                lhsT=kt0[r0, kb0 * 128:(kb0 + 1) * 128],
                rhs=qt0[r0, :],
                start=True, stop=True)
            nc.tensor.matmul(
                ps[:, 512:1024],
                lhsT=kt1[r1, kb1 * 128:(kb1 + 1) * 128],
                rhs=qt1[r1, :],
                start=True, stop=True)
            # exp(s/8 - 4): the -4 shift cancels in softmax and keeps
            # max P ~= e^7 well inside fp16 range
            pt_ = ptp.tile([128, 1024], MD, name="pt_")
            nc.scalar.activation(pt_[:], ps[:], Exp,
                                 scale=0.125, bias=nbias[:])
            pts[n] = pt_

        def stage_pv(n):
            p, hp, c = chunks[n]
            if c == 0:
                pos[p, hp] = ps_o.tile([65, 512], FP, tag="o",
                                       name=f"po{p}_{hp}")
            po = pos[p, hp]
            v = st["v", p]
            pt_ = pts.pop(n)
            kb0, kb1 = 2 * c, 2 * c + 1
            nc.tensor.matmul(
                po[:],
                lhsT=v[:, kb0 * 130 + hp * 65:kb0 * 130 + hp * 65 + 65],
                rhs=pt_[:, 0:512],
                start=(c == 0), stop=False, skip_group_check=True)
            nc.tensor.matmul(
                po[:],
                lhsT=v[:, kb1 * 130 + hp * 65:kb1 * 130 + hp * 65 + 65],
                rhs=pt_[:, 512:1024],
                start=False, stop=(c == 7), skip_group_check=True)
            if c == 7:
                emit_norm(p, hp, pos.pop((p, hp)))

        def interleave(n):
            # next-pair projections and fc-round tiles, one small piece per
            # chunk slot so no emission point exceeds the per-chunk ACT
            # budget (~1.1us)
            p, hp, c = chunks[n]
            t = hp * 8 + c
            if t == 1 and p < 6:
                emit_loads(p + 2)
            elif t == 2 and p < 7:
                emit_kproj(p + 1, 0)
            elif t == 3 and p < 7:
                emit_kproj(p + 1, 1)
            elif t == 4 and p < 7:
                emit_vproj(p + 1, 0)
            elif t == 5 and p < 7:
                emit_vproj(p + 1, 1)
            elif t == 6 and p < 7:
                emit_qproj(p + 1)
            elif 7 <= t <= 14 and p >= 2 and p % 2 == 0:
                emit_fc_tile(p - 2, p - 1, t - 7)

        for n in range(NCH + 1):
            if n < NCH:
                stage_S_exp(n)
            if 0 <= n - 1 < NCH:
                stage_pv(n - 1)
            if n < NCH:
                interleave(n)

        for i in range(4):
            fc_final_early(i)
        for i in range(4):
            fc_final_late(i)
            fc_final_early(i + 4)
        for i in range(4, 8):
            fc_final_late(i)


# ---------------------------------------------------------------------------
# host side
# ---------------------------------------------------------------------------

_NC_CACHE = {}


def _get_nc():
    if KDT not in _NC_CACHE:
        _NC_CACHE[KDT] = build_nc(KDT)
    return _NC_CACHE[KDT]


def _np_dt():
    if KDT == "bf16":
        import ml_dtypes
        return ml_dtypes.bfloat16
    if KDT == "fp16":
        return np.float16
    return np.float32


def _bd(w):
    """128x128 block-diag of W.T (two copies)."""
    wt = np.ascontiguousarray(np.asarray(w).T.astype(np.float32))
    o = np.zeros((128, 128), np.float32)
    o[:64, :64] = wt
    o[64:, 64:] = wt
    return o


def kernel(values, keys, queries, Wv, Wk, Wq, Wo, bo):
    values = np.asarray(values, np.float32)
    keys = np.asarray(keys, np.float32)
    queries = np.asarray(queries, np.float32)

    dt = _np_dt()
    ident = np.eye(128, dtype=np.float32).astype(dt)
    wq_bd = _bd(Wq).astype(dt)
    wk_bd = _bd(Wk).astype(dt)
    wv_bd = _bd(Wv).astype(dt)
    wo_t = np.ascontiguousarray(np.asarray(Wo, np.float32).T).astype(dt)
    bo_r = np.ascontiguousarray(np.asarray(bo, np.float32).reshape(1, E))

    xk_t = [np.ascontiguousarray(keys[b].T).astype(dt) for b in range(B)]
    xv_t = [np.ascontiguousarray(values[b].T).astype(dt) for b in range(B)]

    in_maps = []
    for c in range(NCORES):
        b, qi = c // 4, c % 4
        in_maps.append({
            "xq_t": np.ascontiguousarray(
                queries[b, qi * SQ:(qi + 1) * SQ, :].T).astype(dt),
            "xk_t": xk_t[b],
            "xv_t": xv_t[b],
            "wq_bd": wq_bd, "wk_bd": wk_bd, "wv_bd": wv_bd,
            "wo_t": wo_t, "bo": bo_r, "ident": ident,
        })

    nc = _get_nc()
    res = run_bass_kernel_spmd(nc, in_maps, list(range(NCORES)),
                               trace=bool(int(os.environ.get("BASS_TRACE", "0"))))
    full = np.empty((B, S, E), np.float32)
    for c in range(NCORES):
        b, qi = c // 4, c % 4
        full[b, qi * SQ:(qi + 1) * SQ, :] = res.results[c]["out"]
    kernel.last_results = res
    return full


# revision 32
# speedup vs baseline: 1.1347x; 1.0105x over previous
"""Multi-head attention Trainium2 kernel (8-core SPMD), v2.

Problem: B=2, S=2048, EMBED=1024, HEADS=16, HEAD_DIM=64.
  v,k,q = split_heads(X) @ W{v,k,q}.T  (per-head, shared 64x64 weights)
  out   = softmax(q k^T / 8) v ; merge heads ; out @ Wo.T + bo

Sharding: core c -> batch b=c//4, query rows [qi*512, qi*512+512), qi=c%4.
Each core computes all 16 heads for its 512 query rows; K/V projections are
replicated inside each batch group so NO collectives are needed, and the
output is a disjoint row-slice gather on the host.

v2 design (vs v1 which was PE-bound at ~222us, ACT 64% busy):
  - ACT (exp) is the fundamental floor: 16 heads x 2048k x 512q = 16.7M
    exp elements/core at 1 elem/cycle/lane @1.2GHz ~= 14us/pair busy.
    Everything else is arranged to hide under it.
  - Heads of a pair processed SEQUENTIALLY (not interleaved), and the
    score matmuls (K=64 contraction) are issued as row-tiled pairs: the
    kb-even matmul runs in PE row groups 0-1 (operands at partitions
    0-63) CONCURRENTLY with the kb-odd matmul in row groups 2-3
    (operands at partitions 64-127, ~2x S throughput, measured 3.07x for
    4-way row tiling in the engine docs). This needs the active head's
    K^T/Q^T present in BOTH partition halves: cheap DVE 4x-mode SBUF
    copies (~0.6us/pair) build the duplicate halves.
  - Score PSUM tiles [128,1024] fp32 (2 banks: bank0=kb even, bank1=kb
    odd), double buffered; exp at FD=1024 (~1.15us) back-to-back on ACT.
  - PV unchanged: V_aug [128k, 65] per (kb, head) with a ones column so
    PSUM row 64 accumulates the softmax denominator.
  - Normalization straight from PSUM: reciprocal of po row 64 (cross
    partition-base DVE op), gpsimd partition_broadcast (input must sit at
    partition 0), multiply po rows 0-63 by the broadcast -> merged fp16.
  - fc_out accumulated EVERY 2 PAIRS into SBUF fp32 accumulators (2
    matmuls into a mix PSUM tile + one DVE add), so only et=6,7's
    contribution remains after the last pair -> small tail.
  - PSUM budget: scores 2x2 + po 2 + mix 2 = 8 banks.
"""

import os
import sys

sys.path.insert(0, "/opt/trn_rl_repo")

import numpy as np

import concourse.bass as bass
import concourse.mybir as mybir
import concourse.tile as tile
from concourse import bacc
from concourse.bass_utils import run_bass_kernel_spmd

B = 2
S = 2048
E = 1024
H = 16
D = 64
SQ = 512          # query rows per core
NCORES = 8
NPAIR = 8         # head pairs
KBLK = 16         # 128-row key blocks
FP = mybir.dt.float32

KDT = os.environ.get("KERNEL_DT", "fp16")  # fp16 | bf16 | f32r | fp32


def build_nc(kdt=None):
    kdt = kdt or KDT
    MD = {"fp16": mybir.dt.float16, "bf16": mybir.dt.bfloat16,
          "f32r": mybir.dt.float32r, "fp32": FP}[kdt]  # matmul operand dtype
    nc = bacc.Bacc("TRN2", target_bir_lowering=False, debug=False)

    ident = nc.dram_tensor("ident", [128, 128], MD, kind="ExternalInput").ap()
    xq_t = nc.dram_tensor("xq_t", [E, SQ], MD, kind="ExternalInput").ap()
    xk_t = nc.dram_tensor("xk_t", [E, S], MD, kind="ExternalInput").ap()
    xv_t = nc.dram_tensor("xv_t", [E, S], MD, kind="ExternalInput").ap()
    wq_bd = nc.dram_tensor("wq_bd", [128, 128], MD, kind="ExternalInput").ap()
    wk_bd = nc.dram_tensor("wk_bd", [128, 128], MD, kind="ExternalInput").ap()
    wv_bd = nc.dram_tensor("wv_bd", [128, 128], MD, kind="ExternalInput").ap()
    wo_t = nc.dram_tensor("wo_t", [E, E], MD, kind="ExternalInput").ap()
    bo = nc.dram_tensor("bo", [1, E], FP, kind="ExternalInput").ap()
    out = nc.dram_tensor("out", [SQ, E], FP, kind="ExternalOutput").ap()

    with tile.TileContext(nc) as tc:
        _body(tc, xq_t, xk_t, xv_t, wq_bd, wk_bd, wv_bd, wo_t, bo, ident,
              out, MD)
    nc.compile()
    return nc


def _body(tc, xq_t, xk_t, xv_t, wq_bd, wk_bd, wv_bd, wo_t, bo, ident,
          out, MD):
    """Software-pipelined emission. The Tile scheduler keeps per-engine
    FIFO order ~= emission order, so next-pair DMA loads and projections
    and the fc_out partial rounds are emitted INTERLEAVED into the
    attention chunk stream of the current pair; otherwise they serialize
    at pair boundaries behind the norm chain (observed: 21us ACT stalls
    + PE idle >3.4us -> HAM re-throttle)."""
    from contextlib import ExitStack
    nc = tc.nc
    Exp = mybir.ActivationFunctionType.Exp

    ctx = ExitStack()
    with ctx:
        wp = ctx.enter_context(tc.tile_pool(name="w", bufs=1))
        xkp = ctx.enter_context(tc.tile_pool(name="xk", bufs=3))
        xvp = ctx.enter_context(tc.tile_pool(name="xv", bufs=3))
        xqp = ctx.enter_context(tc.tile_pool(name="xq", bufs=3))
        ktp = ctx.enter_context(tc.tile_pool(name="kt", bufs=2))
        kdp = ctx.enter_context(tc.tile_pool(name="kd", bufs=2))  # dup halves
        vp = ctx.enter_context(tc.tile_pool(name="v", bufs=2))
        qtp = ctx.enter_context(tc.tile_pool(name="qt", bufs=2))
        qdp = ctx.enter_context(tc.tile_pool(name="qd", bufs=2))
        ptp = ctx.enter_context(tc.tile_pool(name="pt", bufs=6))
        mgp = ctx.enter_context(tc.tile_pool(name="mg", bufs=4))
        dnp = ctx.enter_context(tc.tile_pool(name="dn", bufs=4))
        acp = ctx.enter_context(tc.tile_pool(name="ac", bufs=1))
        obp = ctx.enter_context(tc.tile_pool(name="ob", bufs=4))
        ps_s = ctx.enter_context(tc.tile_pool(name="ps_s", bufs=2, space="PSUM"))
        ps_o = ctx.enter_context(tc.tile_pool(name="ps_o", bufs=2, space="PSUM"))
        ps_m = ctx.enter_context(tc.tile_pool(name="ps_m", bufs=2, space="PSUM"))

        # ---- weights / bias; order tuned so the startup critical path
        # (wq -> xq -> qproj, wk -> xk -> kproj -> first S -> first exp)
        # is front-loaded on the single DMA queue ----
        wq = wp.tile([128, 128], MD, tag="wq")
        wk = wp.tile([128, 128], MD, tag="wk")
        wv = wp.tile([128, 128], MD, tag="wv")
        nc.sync.dma_start(wq[:], wq_bd)
        nc.sync.dma_start(wk[:], wk_bd)
        nbias = wp.tile([128, 1], FP, tag="nbias")
        nc.gpsimd.memset(nbias[:], -4.0)
        ones16 = wp.tile([128, KBLK], FP, tag="ones16")
        nc.gpsimd.memset(ones16[:], 1.0)
        # dummy activation: preload the exp table set (~2.7us) during the
        # startup DMAs instead of on the first real exp
        warm = wp.tile([1, 8], FP, tag="warm")
        nc.gpsimd.memset(warm[:], 0.0)
        nc.scalar.activation(warm[:], warm[:], Exp, scale=1.0,
                             bias=nbias[0:1, 0:1])

        wo_tiles = [wp.tile([128, E], MD, tag=f"wo{et}", name=f"wo{et}")
                    for et in range(8)]
        id_t = wp.tile([128, 128], MD, tag="id_t")
        # fc_out SBUF accumulators (fp16 so the final round can fold them
        # into PSUM via an identity-stationary matmul), one per output tile
        acc = [acp.tile([128, 512], MD, tag=f"acc{i}", name=f"acc{i}")
               for i in range(8)]
        merged = {}
        st = {}  # pipelined per-pair tiles

        def emit_loads(p):
            xq = xqp.tile([128, SQ], MD, tag="xq")
            nc.sync.dma_start(xq[:], xq_t[p * 128:(p + 1) * 128, :])
            xk = xkp.tile([128, S], MD, tag="xk")
            for ch in range(4):
                nc.sync.dma_start(xk[:, ch * 512:(ch + 1) * 512],
                                  xk_t[p * 128:(p + 1) * 128,
                                       ch * 512:(ch + 1) * 512])
            xv = xvp.tile([128, S], MD, tag="xv")
            for ch in range(2):
                nc.sync.dma_start(xv[:, ch * 1024:(ch + 1) * 1024],
                                  xv_t[p * 128:(p + 1) * 128,
                                       ch * 1024:(ch + 1) * 1024])
            st["x", p] = (xk, xv, xq)

        def emit_kproj(p, half, dups=True):
            # K^T projection [128(d2), 2048(k)] + chunked dup halves so the
            # active head's K^T exists at BOTH partition halves for the
            # row-tiled S pairs. Pair 0 skips the dups (its S matmuls run
            # serially in one row group) to keep the startup DVE chain short.
            xk = st["x", p][0]
            if half == 0:
                kt = ktp.tile([128, S], MD, tag="kt")
                if dups:
                    kdA = kdp.tile([128, S], MD, tag="kdA")  # even @ 64:128
                    kdB = kdp.tile([128, S], MD, tag="kdB")  # odd @ 0:64
                else:
                    kdA = kdB = None
                st["k", p] = (kt, kdA, kdB)
            else:
                kt, kdA, kdB = st["k", p]
            for ch in (2 * half, 2 * half + 1):
                sl = slice(ch * 512, (ch + 1) * 512)
                ps = ps_m.tile([128, 512], FP, tag="mix")
                nc.tensor.matmul(ps[:], lhsT=wk[:], rhs=xk[:, sl],
                                 start=True, stop=True)
                nc.vector.tensor_copy(kt[:, sl], ps[:])
                if dups:
                    nc.vector.tensor_copy(kdA[64:128, sl], kt[0:64, sl])
                    nc.vector.tensor_copy(kdB[0:64, sl], kt[64:128, sl])

        def emit_vproj(p, half):
            # V natural projection with ones columns (col 64 of each head
            # block, so the softmax denominator lands in po row 64)
            xv = st["x", p][1]
            if half == 0:
                v = vp.tile([128, KBLK * 130], MD, tag="v")
                vr = v[:].rearrange("p (b c) -> p b c", c=130)
                nc.vector.tensor_copy(vr[:, :, 64:65], ones16[:])
                nc.vector.tensor_copy(vr[:, :, 129:130], ones16[:])
                st["v", p] = v
            else:
                v = st["v", p]
            for vg in (2 * half, 2 * half + 1):
                ps = ps_m.tile([128, 512], FP, tag="mix")
                for j in range(4):
                    kb = vg * 4 + j
                    nc.tensor.matmul(ps[:, j * 128:(j + 1) * 128],
                                     lhsT=xv[:, kb * 128:(kb + 1) * 128],
                                     rhs=wv[:], start=True, stop=True)
                src4 = ps[:].rearrange("p (b g c) -> p b g c", g=2, c=64)
                dst4 = v[:, vg * 520:(vg + 1) * 520].rearrange(
                    "p (b g c) -> p b g c", g=2, c=65)[:, :, :, 0:64]
                nc.vector.tensor_copy(dst4, src4)

        def emit_qproj(p, dups=True):
            xq = st["x", p][2]
            qt = qtp.tile([128, SQ], MD, tag="qt")
            psq = ps_m.tile([128, 512], FP, tag="mix")
            nc.tensor.matmul(psq[:], lhsT=wq[:], rhs=xq[:],
                             start=True, stop=True)
            nc.vector.tensor_copy(qt[:], psq[:])
            if dups:
                qdA = qdp.tile([128, SQ], MD, tag="qdA")
                nc.vector.tensor_copy(qdA[64:128, :], qt[0:64, :])
                qdB = qdp.tile([128, SQ], MD, tag="qdB")
                nc.vector.tensor_copy(qdB[0:64, :], qt[64:128, :])
            else:
                qdA = qdB = None
            st["q", p] = (qt, qdA, qdB)

        def emit_fc_tile(pa, pb, i):
            # one fc_out output tile: acc[i] (+)= merged[pa] @ wo[pa]
            #                                  + merged[pb] @ wo[pb]
            sb, nch = i // 2, i % 2
            psf_t = ps_m.tile([128, 512], FP, tag="mix", name="psf")
            psf = psf_t[:]
            nc.tensor.matmul(
                psf,
                lhsT=merged[pa][:, sb * 128:(sb + 1) * 128],
                rhs=wo_tiles[pa][:, nch * 512:(nch + 1) * 512],
                start=True, stop=False, skip_group_check=True)
            nc.tensor.matmul(
                psf,
                lhsT=merged[pb][:, sb * 128:(sb + 1) * 128],
                rhs=wo_tiles[pb][:, nch * 512:(nch + 1) * 512],
                start=False, stop=True, skip_group_check=True)
            if pa == 0:
                nc.vector.tensor_add(acc[i][:], psf,
                                     bo_b[:, nch * 512:(nch + 1) * 512])
            else:
                nc.vector.tensor_add(acc[i][:], acc[i][:], psf)

        # final fc round, split so the norm(7,hp1)-independent matmuls
        # (merged[6], merged[7] top half, identity*acc) keep the PE warm
        # while the last norm chain runs on DVE/GpSimd; evacuation
        # alternates ACT (idle after the last exp) and DVE.
        psfs = {}

        def fc_final_early(i):
            sb, nch = i // 2, i % 2
            if i % 2 == 0:
                big = ps_s.tile([128, 1024], FP, tag="s", name=f"fcf{i}")
                psf = big[:, 0:512]
            else:
                psf_t = ps_m.tile([128, 512], FP, tag="mix", name="psf")
                psf = psf_t[:]
            nc.tensor.matmul(
                psf, lhsT=merged[6][:, sb * 128:(sb + 1) * 128],
                rhs=wo_tiles[6][:, nch * 512:(nch + 1) * 512],
                start=True, stop=False, skip_group_check=True)
            nc.tensor.matmul(
                psf, lhsT=merged[7][0:64, sb * 128:(sb + 1) * 128],
                rhs=wo_tiles[7][0:64, nch * 512:(nch + 1) * 512],
                start=False, stop=False, skip_group_check=True)
            nc.tensor.matmul(
                psf, lhsT=id_t[:], rhs=acc[i][:],
                start=False, stop=False, skip_group_check=True)
            psfs[i] = psf

        def fc_final_late(i):
            sb, nch = i // 2, i % 2
            psf = psfs.pop(i)
            nc.tensor.matmul(
                psf, lhsT=merged[7][64:128, sb * 128:(sb + 1) * 128],
                rhs=wo_tiles[7][64:128, nch * 512:(nch + 1) * 512],
                start=False, stop=True, skip_group_check=True)
            ot = obp.tile([128, 512], FP, tag="ob")
            if i % 2 == 0:
                nc.scalar.copy(ot[:], psf)
            else:
                nc.vector.tensor_copy(ot[:], psf)
            nc.sync.dma_start(
                out[sb * 128:(sb + 1) * 128,
                    nch * 512:(nch + 1) * 512], ot[:])

        def emit_norm(p, hp, po):
            # normalize: denominator row 64 -> partition 0 via a standard
            # copy (honors AP partition offsets; custom-DVE recip and
            # gpsimd broadcast need input physically at partition 0),
            # then multiply po rows 0-63 straight from PSUM (base 0).
            mg = mgp.tile([128, SQ], MD, name=f"m{p}", tag="mg") \
                if hp == 0 else merged[p]
            merged[p] = mg
            dn = dnp.tile([1, 512], FP, tag="dn")
            nc.vector.tensor_copy(dn[0:1, :], po[64:65, :])
            dr = dnp.tile([1, 512], FP, tag="dr")
            nc.vector.reciprocal_approx_fast(dr[0:1, :], dn[0:1, :])
            db = dnp.tile([64, 512], FP, tag="db")
            nc.gpsimd.partition_broadcast(db[:], dr[0:1, :], channels=64)
            nc.vector.tensor_mul(mg[hp * 64:(hp + 1) * 64, :],
                                 po[0:64, :], db[:])

        # ---- prologue: pair 0 fully, pair 1 loads ----
        emit_loads(0)
        nc.sync.dma_start(wv[:], wv_bd)
        bo_row = wp.tile([1, E], FP, tag="bo_row")
        nc.sync.dma_start(bo_row[:], bo)
        bo_b = wp.tile([128, E], FP, tag="bo_b")
        nc.gpsimd.partition_broadcast(bo_b[:], bo_row[0:1, :], channels=128)
        emit_qproj(0, dups=False)
        emit_kproj(0, 0, dups=False)
        emit_kproj(0, 1, dups=False)
        # pair-0 B-side dups only: hp0 runs serial S (PE is HAM-cold anyway)
        # but hp1 (past the cold window) still row-tiles
        kt0_ = st["k", 0][0]
        qt0_ = st["q", 0][0]
        kdB0 = kdp.tile([128, S], MD, tag="kdB")
        nc.vector.tensor_copy(kdB0[0:64, :], kt0_[64:128, :])
        qdB0 = qdp.tile([128, SQ], MD, tag="qdB")
        nc.vector.tensor_copy(qdB0[0:64, :], qt0_[64:128, :])
        st["k", 0] = (kt0_, None, kdB0)
        st["q", 0] = (qt0_, None, qdB0)
        emit_vproj(0, 0)
        emit_vproj(0, 1)
        emit_loads(1)

        # ---- attention: one flat chunk stream, software-pipelined ONE
        # deep (S+exp at n, PV at n-1) - the next chunk's S pair enters the
        # PE FIFO ahead of the previous chunk's PV, keeping ACT fed across
        # chunk, head, and pair boundaries. (A two-deep variant measured
        # WORSE: exp durations grew 1113->1335ns from the extra concurrent
        # PSUM traffic.) Heads sequential; S matmuls issued as row-tiled
        # pairs (kb even in PE rows 0-63, kb odd in 64-127, concurrent on
        # HW); exp FD=1024; PV accumulates po.
        chunks = [(p, hp, c) for p in range(NPAIR) for hp in range(2)
                  for c in range(8)]
        NCH = len(chunks)
        pts, pos = {}, {}

        def stage_S_exp(n):
            p, hp, c = chunks[n]
            if p == 1 and hp == 0 and c == 0:
                # fc weights aren't needed until pair 2's fc round
                for et in range(8):
                    nc.sync.dma_start(wo_tiles[et][:],
                                      wo_t[et * 128:(et + 1) * 128, :])
                nc.sync.dma_start(id_t[:], ident)
            kt, kdA, kdB = st["k", p]
            qt, qdA, qdB = st["q", p]
            if kdA is None and hp == 0:
                # pair 0 head 0: no A-dups; both S matmuls in row group 0
                # (serial on PE - fine, startup is HAM-cold anyway)
                r0 = r1 = slice(0, 64)
                kt0 = kt1 = kt
                qt0 = qt1 = qt
            elif hp == 0:
                kt0, qt0, r0 = kt, qt, slice(0, 64)
                kt1, qt1, r1 = kdA, qdA, slice(64, 128)
            else:
                kt0, qt0, r0 = kdB, qdB, slice(0, 64)
                kt1, qt1, r1 = kt, qt, slice(64, 128)
            kb0, kb1 = 2 * c, 2 * c + 1
            ps = ps_s.tile([128, 1024], FP, tag="s", name=f"s{p}_{hp}_{c}")
            nc.tensor.matmul(
                ps[:, 0:512],
                lhsT=kt0[r0, kb0 * 128:(kb0 + 1) * 128],
                rhs=qt0[r0, :],
                start=True, stop=True)
            nc.tensor.matmul(
                ps[:, 512:1024],
                lhsT=kt1[r1, kb1 * 128:(kb1 + 1) * 128],
                rhs=qt1[r1, :],
                start=True, stop=True)
            # exp(s/8 - 4): the -4 shift cancels in softmax and keeps
            # max P ~= e^7 well inside fp16 range
            pt_ = ptp.tile([128, 1024], MD, name="pt_")
            nc.scalar.activation(pt_[:], ps[:], Exp,
                                 scale=0.125, bias=nbias[:])
            pts[n] = pt_

        def stage_pv(n):
            p, hp, c = chunks[n]
            if c == 0:
                pos[p, hp] = ps_o.tile([65, 512], FP, tag="o",
                                       name=f"po{p}_{hp}")
            po = pos[p, hp]
            v = st["v", p]
            pt_ = pts.pop(n)
            kb0, kb1 = 2 * c, 2 * c + 1
            nc.tensor.matmul(
                po[:],
                lhsT=v[:, kb0 * 130 + hp * 65:kb0 * 130 + hp * 65 + 65],
                rhs=pt_[:, 0:512],
                start=(c == 0), stop=False, skip_group_check=True)
            nc.tensor.matmul(
                po[:],
                lhsT=v[:, kb1 * 130 + hp * 65:kb1 * 130 + hp * 65 + 65],
                rhs=pt_[:, 512:1024],
                start=False, stop=(c == 7), skip_group_check=True)
            if c == 7:
                emit_norm(p, hp, pos.pop((p, hp)))

        def interleave(n):
            # next-pair projections and fc-round tiles, one small piece per
            # chunk slot so no emission point exceeds the per-chunk ACT
            # budget (~1.1us)
            p, hp, c = chunks[n]
            t = hp * 8 + c
            if t == 1 and p < 6:
                emit_loads(p + 2)
                return
            # pair 0 defers its projection slots past the HAM-cold window
            tt = t - 4 if p == 0 else t
            if tt == 2 and p < 7:
                emit_kproj(p + 1, 0)
            elif tt == 3 and p < 7:
                emit_kproj(p + 1, 1)
            elif tt == 4 and p < 7:
                emit_vproj(p + 1, 0)
            elif tt == 5 and p < 7:
                emit_vproj(p + 1, 1)
            elif tt == 6 and p < 7:
                emit_qproj(p + 1)
            elif 7 <= t <= 10 and p >= 2:
                # fc round for an earlier pair couple, 4 tiles on the even
                # pair and 4 on the odd pair, so no single pair carries the
                # whole 16-matmul round
                if p % 2 == 0:
                    emit_fc_tile(p - 2, p - 1, t - 7)
                else:
                    emit_fc_tile(p - 3, p - 2, t - 3)

        for n in range(NCH + 1):
            if n < NCH:
                stage_S_exp(n)
            if 0 <= n - 1 < NCH:
                stage_pv(n - 1)
            if n < NCH:
                interleave(n)

        for i in range(4):
            fc_final_early(i)
        for i in range(4):
            fc_final_late(i)
            fc_final_early(i + 4)
        for i in range(4, 8):
            fc_final_late(i)


# ---------------------------------------------------------------------------
# host side
# ---------------------------------------------------------------------------

_NC_CACHE = {}


def _get_nc():
    if KDT not in _NC_CACHE:
        _NC_CACHE[KDT] = build_nc(KDT)
    return _NC_CACHE[KDT]


def _np_dt():
    if KDT == "bf16":
        import ml_dtypes
        return ml_dtypes.bfloat16
    if KDT == "fp16":
        return np.float16
    return np.float32


def _bd(w):
    """128x128 block-diag of W.T (two copies)."""
    wt = np.ascontiguousarray(np.asarray(w).T.astype(np.float32))
    o = np.zeros((128, 128), np.float32)
    o[:64, :64] = wt
    o[64:, 64:] = wt
    return o


def kernel(values, keys, queries, Wv, Wk, Wq, Wo, bo):
    values = np.asarray(values, np.float32)
    keys = np.asarray(keys, np.float32)
    queries = np.asarray(queries, np.float32)

    dt = _np_dt()
    ident = np.eye(128, dtype=np.float32).astype(dt)
    wq_bd = _bd(Wq).astype(dt)
    wk_bd = _bd(Wk).astype(dt)
    wv_bd = _bd(Wv).astype(dt)
    wo_t = np.ascontiguousarray(np.asarray(Wo, np.float32).T).astype(dt)
    bo_r = np.ascontiguousarray(np.asarray(bo, np.float32).reshape(1, E))

    xk_t = [np.ascontiguousarray(keys[b].T).astype(dt) for b in range(B)]
    xv_t = [np.ascontiguousarray(values[b].T).astype(dt) for b in range(B)]

    in_maps = []
    for c in range(NCORES):
        b, qi = c // 4, c % 4
        in_maps.append({
            "xq_t": np.ascontiguousarray(
                queries[b, qi * SQ:(qi + 1) * SQ, :].T).astype(dt),
            "xk_t": xk_t[b],
            "xv_t": xv_t[b],
            "wq_bd": wq_bd, "wk_bd": wk_bd, "wv_bd": wv_bd,
            "wo_t": wo_t, "bo": bo_r, "ident": ident,
        })

    nc = _get_nc()
    res = run_bass_kernel_spmd(nc, in_maps, list(range(NCORES)),
                               trace=bool(int(os.environ.get("BASS_TRACE", "0"))))
    full = np.empty((B, S, E), np.float32)
    for c in range(NCORES):
        b, qi = c // 4, c % 4
        full[b, qi * SQ:(qi + 1) * SQ, :] = res.results[c]["out"]
    kernel.last_results = res
    return full


# revision 34
# speedup vs baseline: 1.1422x; 1.0066x over previous
"""Multi-head attention Trainium2 kernel (8-core SPMD).

Problem: B=2, S=2048, EMBED=1024, HEADS=16, HEAD_DIM=64.
  v,k,q = split_heads(X) @ W{v,k,q}.T  (per-head, shared 64x64 weights)
  out   = softmax(q k^T / 8) v ; merge heads ; out @ Wo.T + bo

Sharding: core c -> batch b=c//4, query rows [qi*512, qi*512+512), qi=c%4.
Each core computes all 16 heads for its 512 query rows; K/V projections are
replicated inside each batch group so NO collectives are needed, and the
output is a disjoint row-slice gather on the host.

Design (v1 was PE-bound at ~222us; this version is ACT-bound, the
fundamental floor for this problem):
  - ACT (exp) floor: 16 heads x 2048k x 512q = 16.7M exp elements/core
    at 1 elem/cycle/lane @1.2GHz = ~143us busy (FD=1024 instructions,
    (N+318)/1.2ns each). Everything else hides under it.
  - Heads of a pair processed SEQUENTIALLY; the K=64-contraction score
    matmuls are issued as row-tiled pairs (kb-even in PE row groups 0-1,
    kb-odd concurrently in groups 2-3; operands live at the matching
    partition halves via cheap DVE 4x-mode dup copies). ~2x S throughput.
  - One flat chunk stream, software-pipelined one deep (S+exp at n, PV
    at n-1) so the next chunk's S pair sits in the PE FIFO ahead of the
    previous chunk's PV; ACT stays fed across chunk/head/pair boundaries.
    (Two deep measured WORSE: exp slowed 1113->1335ns from extra
    concurrent PSUM traffic.)
  - Next-pair DMA loads + K/V/Q projections and the fc_out partial
    rounds are emitted in small pieces at fixed chunk slots (the Tile
    scheduler keeps per-engine FIFO order ~= emission order; lumping
    them stalls the PE FIFO and starves ACT).
  - V_aug [128k, 65] per (kb, head) carries a ones column so PSUM row 64
    accumulates the softmax denominator during PV. Normalization:
    denominator row -> partition 0 (standard DVE copy honors partition
    offsets), custom-DVE reciprocal, gpsimd partition_broadcast,
    multiply straight from PSUM rows 0-63 -> merged fp16.
  - fc_out: partial rounds every 2 pairs (2 MMs into a mix PSUM tile +
    DVE add into fp16 SBUF accumulators, 4 tiles on each of 2 pairs);
    the final round folds acc back in via an identity-stationary matmul
    and evacuates alternately on ACT (idle once exps end) and DVE, with
    the norm-independent matmuls emitted first to keep the PE warm.
  - Startup: exp ACT-table preloaded via a dummy activation; ~4us of
    junk fp32 matmuls during the DMA wait flip the HAM clock gate to
    8/8 before the first real matmul; pair 0 skips the A-side dup tiles
    (serial S in one row group) to shorten the critical DVE chain.
  - PSUM budget: scores 2x[128,1024]f32 (4 banks) + po 2 + mix 2 = 8.
"""

import os
import sys

sys.path.insert(0, "/opt/trn_rl_repo")

import numpy as np

import concourse.bass as bass
import concourse.mybir as mybir
import concourse.tile as tile
from concourse import bacc
from concourse.bass_utils import run_bass_kernel_spmd

B = 2
S = 2048
E = 1024
H = 16
D = 64
SQ = 512          # query rows per core
NCORES = 8
NPAIR = 8         # head pairs
KBLK = 16         # 128-row key blocks
FP = mybir.dt.float32

KDT = os.environ.get("KERNEL_DT", "fp16")  # fp16 | bf16 | f32r | fp32


def build_nc(kdt=None):
    kdt = kdt or KDT
    MD = {"fp16": mybir.dt.float16, "bf16": mybir.dt.bfloat16,
          "f32r": mybir.dt.float32r, "fp32": FP}[kdt]  # matmul operand dtype
    nc = bacc.Bacc("TRN2", target_bir_lowering=False, debug=False)

    ident = nc.dram_tensor("ident", [128, 128], MD, kind="ExternalInput").ap()
    xq_t = nc.dram_tensor("xq_t", [E, SQ], MD, kind="ExternalInput").ap()
    xk_t = nc.dram_tensor("xk_t", [E, S], MD, kind="ExternalInput").ap()
    xv_t = nc.dram_tensor("xv_t", [E, S], MD, kind="ExternalInput").ap()
    wq_bd = nc.dram_tensor("wq_bd", [128, 128], MD, kind="ExternalInput").ap()
    wk_bd = nc.dram_tensor("wk_bd", [128, 128], MD, kind="ExternalInput").ap()
    wv_bd = nc.dram_tensor("wv_bd", [128, 128], MD, kind="ExternalInput").ap()
    wo_t = nc.dram_tensor("wo_t", [E, E], MD, kind="ExternalInput").ap()
    bo = nc.dram_tensor("bo", [1, E], FP, kind="ExternalInput").ap()
    out = nc.dram_tensor("out", [SQ, E], FP, kind="ExternalOutput").ap()

    with tile.TileContext(nc) as tc:
        _body(tc, xq_t, xk_t, xv_t, wq_bd, wk_bd, wv_bd, wo_t, bo, ident,
              out, MD)
    nc.compile()
    return nc


def _body(tc, xq_t, xk_t, xv_t, wq_bd, wk_bd, wv_bd, wo_t, bo, ident,
          out, MD):
    """Software-pipelined emission. The Tile scheduler keeps per-engine
    FIFO order ~= emission order, so next-pair DMA loads and projections
    and the fc_out partial rounds are emitted INTERLEAVED into the
    attention chunk stream of the current pair; otherwise they serialize
    at pair boundaries behind the norm chain (observed: 21us ACT stalls
    + PE idle >3.4us -> HAM re-throttle)."""
    from contextlib import ExitStack
    nc = tc.nc
    Exp = mybir.ActivationFunctionType.Exp

    ctx = ExitStack()
    with ctx:
        wp = ctx.enter_context(tc.tile_pool(name="w", bufs=1))
        xkp = ctx.enter_context(tc.tile_pool(name="xk", bufs=3))
        xvp = ctx.enter_context(tc.tile_pool(name="xv", bufs=3))
        xqp = ctx.enter_context(tc.tile_pool(name="xq", bufs=3))
        ktp = ctx.enter_context(tc.tile_pool(name="kt", bufs=2))
        kdp = ctx.enter_context(tc.tile_pool(name="kd", bufs=2))  # dup halves
        vp = ctx.enter_context(tc.tile_pool(name="v", bufs=2))
        qtp = ctx.enter_context(tc.tile_pool(name="qt", bufs=2))
        qdp = ctx.enter_context(tc.tile_pool(name="qd", bufs=2))
        ptp = ctx.enter_context(tc.tile_pool(name="pt", bufs=6))
        mgp = ctx.enter_context(tc.tile_pool(name="mg", bufs=4))
        dnp = ctx.enter_context(tc.tile_pool(name="dn", bufs=4))
        acp = ctx.enter_context(tc.tile_pool(name="ac", bufs=1))
        obp = ctx.enter_context(tc.tile_pool(name="ob", bufs=4))
        ps_s = ctx.enter_context(tc.tile_pool(name="ps_s", bufs=2, space="PSUM"))
        ps_o = ctx.enter_context(tc.tile_pool(name="ps_o", bufs=2, space="PSUM"))
        ps_m = ctx.enter_context(tc.tile_pool(name="ps_m", bufs=2, space="PSUM"))

        # ---- weights / bias; order tuned so the startup critical path
        # (wq -> xq -> qproj, wk -> xk -> kproj -> first S -> first exp)
        # is front-loaded on the single DMA queue ----
        wq = wp.tile([128, 128], MD, tag="wq")
        wk = wp.tile([128, 128], MD, tag="wk")
        wv = wp.tile([128, 128], MD, tag="wv")
        nc.sync.dma_start(wq[:], wq_bd)
        nc.sync.dma_start(wk[:], wk_bd)
        nbias = wp.tile([128, 1], FP, tag="nbias")
        nc.gpsimd.memset(nbias[:], -4.0)
        ones16 = wp.tile([128, KBLK], FP, tag="ones16")
        nc.gpsimd.memset(ones16[:], 1.0)
        # dummy activation: preload the exp table set (~2.7us) during the
        # startup DMAs instead of on the first real exp
        warm = wp.tile([1, 8], FP, tag="warm")
        nc.gpsimd.memset(warm[:], 0.0)
        nc.scalar.activation(warm[:], warm[:], Exp, scale=1.0,
                             bias=nbias[0:1, 0:1])
        # dummy fp32 matmuls on junk data: ~4us of sustained PE activity
        # during the startup DMA wait flips the HAM clock gate to 8/8, so
        # the first REAL matmuls run at 2.4GHz instead of 1.2
        wmt = wp.tile([128, 512], FP, tag="wmt")
        nc.gpsimd.memset(wmt[:], 0.25)
        for w_ in range(5):
            psw = ps_m.tile([128, 512], FP, tag="mix", name="psw")
            nc.tensor.matmul(psw[:], lhsT=wmt[:, 0:128], rhs=wmt[:],
                             start=True, stop=True)

        wo_tiles = [wp.tile([128, E], MD, tag=f"wo{et}", name=f"wo{et}")
                    for et in range(8)]
        id_t = wp.tile([128, 128], MD, tag="id_t")
        # fc_out SBUF accumulators (fp16 so the final round can fold them
        # into PSUM via an identity-stationary matmul), one per output tile
        acc = [acp.tile([128, 512], MD, tag=f"acc{i}", name=f"acc{i}")
               for i in range(8)]
        merged = {}
        st = {}  # pipelined per-pair tiles

        def emit_loads(p):
            xq = xqp.tile([128, SQ], MD, tag="xq")
            nc.sync.dma_start(xq[:], xq_t[p * 128:(p + 1) * 128, :])
            xk = xkp.tile([128, S], MD, tag="xk")
            for ch in range(4):
                nc.sync.dma_start(xk[:, ch * 512:(ch + 1) * 512],
                                  xk_t[p * 128:(p + 1) * 128,
                                       ch * 512:(ch + 1) * 512])
            xv = xvp.tile([128, S], MD, tag="xv")
            for ch in range(2):
                nc.sync.dma_start(xv[:, ch * 1024:(ch + 1) * 1024],
                                  xv_t[p * 128:(p + 1) * 128,
                                       ch * 1024:(ch + 1) * 1024])
            st["x", p] = (xk, xv, xq)

        def emit_kproj(p, half, dups=True):
            # K^T projection [128(d2), 2048(k)] + chunked dup halves so the
            # active head's K^T exists at BOTH partition halves for the
            # row-tiled S pairs. Pair 0 skips the dups (its S matmuls run
            # serially in one row group) to keep the startup DVE chain short.
            xk = st["x", p][0]
            if half == 0:
                kt = ktp.tile([128, S], MD, tag="kt")
                if dups:
                    kdA = kdp.tile([128, S], MD, tag="kdA")  # even @ 64:128
                    kdB = kdp.tile([128, S], MD, tag="kdB")  # odd @ 0:64
                else:
                    kdA = kdB = None
                st["k", p] = (kt, kdA, kdB)
            else:
                kt, kdA, kdB = st["k", p]
            for ch in (2 * half, 2 * half + 1):
                sl = slice(ch * 512, (ch + 1) * 512)
                ps = ps_m.tile([128, 512], FP, tag="mix")
                nc.tensor.matmul(ps[:], lhsT=wk[:], rhs=xk[:, sl],
                                 start=True, stop=True)
                nc.vector.tensor_copy(kt[:, sl], ps[:])
                if dups:
                    nc.vector.tensor_copy(kdA[64:128, sl], kt[0:64, sl])
                    nc.vector.tensor_copy(kdB[0:64, sl], kt[64:128, sl])

        def emit_vproj(p, half):
            # V natural projection with ones columns (col 64 of each head
            # block, so the softmax denominator lands in po row 64)
            xv = st["x", p][1]
            if half == 0:
                v = vp.tile([128, KBLK * 130], MD, tag="v")
                vr = v[:].rearrange("p (b c) -> p b c", c=130)
                nc.vector.tensor_copy(vr[:, :, 64:65], ones16[:])
                nc.vector.tensor_copy(vr[:, :, 129:130], ones16[:])
                st["v", p] = v
            else:
                v = st["v", p]
            for vg in (2 * half, 2 * half + 1):
                ps = ps_m.tile([128, 512], FP, tag="mix")
                for j in range(4):
                    kb = vg * 4 + j
                    nc.tensor.matmul(ps[:, j * 128:(j + 1) * 128],
                                     lhsT=xv[:, kb * 128:(kb + 1) * 128],
                                     rhs=wv[:], start=True, stop=True)
                src4 = ps[:].rearrange("p (b g c) -> p b g c", g=2, c=64)
                dst4 = v[:, vg * 520:(vg + 1) * 520].rearrange(
                    "p (b g c) -> p b g c", g=2, c=65)[:, :, :, 0:64]
                nc.vector.tensor_copy(dst4, src4)

        def emit_qproj(p, dups=True):
            xq = st["x", p][2]
            qt = qtp.tile([128, SQ], MD, tag="qt")
            psq = ps_m.tile([128, 512], FP, tag="mix")
            nc.tensor.matmul(psq[:], lhsT=wq[:], rhs=xq[:],
                             start=True, stop=True)
            nc.vector.tensor_copy(qt[:], psq[:])
            if dups:
                qdA = qdp.tile([128, SQ], MD, tag="qdA")
                nc.vector.tensor_copy(qdA[64:128, :], qt[0:64, :])
                qdB = qdp.tile([128, SQ], MD, tag="qdB")
                nc.vector.tensor_copy(qdB[0:64, :], qt[64:128, :])
            else:
                qdA = qdB = None
            st["q", p] = (qt, qdA, qdB)

        def emit_fc_tile(pa, pb, i):
            # one fc_out output tile: acc[i] (+)= merged[pa] @ wo[pa]
            #                                  + merged[pb] @ wo[pb]
            sb, nch = i // 2, i % 2
            psf_t = ps_m.tile([128, 512], FP, tag="mix", name="psf")
            psf = psf_t[:]
            nc.tensor.matmul(
                psf,
                lhsT=merged[pa][:, sb * 128:(sb + 1) * 128],
                rhs=wo_tiles[pa][:, nch * 512:(nch + 1) * 512],
                start=True, stop=False, skip_group_check=True)
            nc.tensor.matmul(
                psf,
                lhsT=merged[pb][:, sb * 128:(sb + 1) * 128],
                rhs=wo_tiles[pb][:, nch * 512:(nch + 1) * 512],
                start=False, stop=True, skip_group_check=True)
            if pa == 0:
                nc.vector.tensor_add(acc[i][:], psf,
                                     bo_b[:, nch * 512:(nch + 1) * 512])
            else:
                nc.vector.tensor_add(acc[i][:], acc[i][:], psf)

        # final fc round, split so the norm(7,hp1)-independent matmuls
        # (merged[6], merged[7] top half, identity*acc) keep the PE warm
        # while the last norm chain runs on DVE/GpSimd; evacuation
        # alternates ACT (idle after the last exp) and DVE.
        psfs = {}

        def fc_final_early(i):
            sb, nch = i // 2, i % 2
            if i % 2 == 0:
                big = ps_s.tile([128, 1024], FP, tag="s", name=f"fcf{i}")
                psf = big[:, 0:512]
            else:
                psf_t = ps_m.tile([128, 512], FP, tag="mix", name="psf")
                psf = psf_t[:]
            nc.tensor.matmul(
                psf, lhsT=merged[6][:, sb * 128:(sb + 1) * 128],
                rhs=wo_tiles[6][:, nch * 512:(nch + 1) * 512],
                start=True, stop=False, skip_group_check=True)
            nc.tensor.matmul(
                psf, lhsT=merged[7][0:64, sb * 128:(sb + 1) * 128],
                rhs=wo_tiles[7][0:64, nch * 512:(nch + 1) * 512],
                start=False, stop=False, skip_group_check=True)
            nc.tensor.matmul(
                psf, lhsT=id_t[:], rhs=acc[i][:],
                start=False, stop=False, skip_group_check=True)
            psfs[i] = psf

        def fc_final_late(i):
            sb, nch = i // 2, i % 2
            psf = psfs.pop(i)
            nc.tensor.matmul(
                psf, lhsT=merged[7][64:128, sb * 128:(sb + 1) * 128],
                rhs=wo_tiles[7][64:128, nch * 512:(nch + 1) * 512],
                start=False, stop=True, skip_group_check=True)
            ot = obp.tile([128, 512], FP, tag="ob")
            if i % 2 == 0:
                nc.scalar.copy(ot[:], psf)
            else:
                nc.vector.tensor_copy(ot[:], psf)
            nc.sync.dma_start(
                out[sb * 128:(sb + 1) * 128,
                    nch * 512:(nch + 1) * 512], ot[:])

        def emit_norm(p, hp, po):
            # normalize: denominator row 64 -> partition 0 via a standard
            # copy (honors AP partition offsets; custom-DVE recip and
            # gpsimd broadcast need input physically at partition 0),
            # then multiply po rows 0-63 straight from PSUM (base 0).
            mg = mgp.tile([128, SQ], MD, name=f"m{p}", tag="mg") \
                if hp == 0 else merged[p]
            merged[p] = mg
            dn = dnp.tile([1, 512], FP, tag="dn")
            nc.vector.tensor_copy(dn[0:1, :], po[64:65, :])
            dr = dnp.tile([1, 512], FP, tag="dr")
            nc.vector.reciprocal_approx_fast(dr[0:1, :], dn[0:1, :])
            db = dnp.tile([64, 512], FP, tag="db")
            nc.gpsimd.partition_broadcast(db[:], dr[0:1, :], channels=64)
            nc.vector.tensor_mul(mg[hp * 64:(hp + 1) * 64, :],
                                 po[0:64, :], db[:])

        # ---- prologue: pair 0 fully, pair 1 loads ----
        emit_loads(0)
        nc.sync.dma_start(wv[:], wv_bd)
        bo_row = wp.tile([1, E], FP, tag="bo_row")
        nc.sync.dma_start(bo_row[:], bo)
        bo_b = wp.tile([128, E], FP, tag="bo_b")
        nc.gpsimd.partition_broadcast(bo_b[:], bo_row[0:1, :], channels=128)
        emit_qproj(0, dups=False)
        emit_kproj(0, 0, dups=False)
        emit_kproj(0, 1, dups=False)
        # pair-0 B-side dups only: hp0 runs serial S (PE is HAM-cold anyway)
        # but hp1 (past the cold window) still row-tiles
        kt0_ = st["k", 0][0]
        qt0_ = st["q", 0][0]
        kdB0 = kdp.tile([128, S], MD, tag="kdB")
        nc.vector.tensor_copy(kdB0[0:64, :], kt0_[64:128, :])
        qdB0 = qdp.tile([128, SQ], MD, tag="qdB")
        nc.vector.tensor_copy(qdB0[0:64, :], qt0_[64:128, :])
        st["k", 0] = (kt0_, None, kdB0)
        st["q", 0] = (qt0_, None, qdB0)
        emit_vproj(0, 0)
        emit_vproj(0, 1)
        emit_loads(1)

        # ---- attention: one flat chunk stream, software-pipelined ONE
        # deep (S+exp at n, PV at n-1) - the next chunk's S pair enters the
        # PE FIFO ahead of the previous chunk's PV, keeping ACT fed across
        # chunk, head, and pair boundaries. (A two-deep variant measured
        # WORSE: exp durations grew 1113->1335ns from the extra concurrent
        # PSUM traffic.) Heads sequential; S matmuls issued as row-tiled
        # pairs (kb even in PE rows 0-63, kb odd in 64-127, concurrent on
        # HW); exp FD=1024; PV accumulates po.
        chunks = [(p, hp, c) for p in range(NPAIR) for hp in range(2)
                  for c in range(8)]
        NCH = len(chunks)
        pts, pos = {}, {}

        def stage_S_exp(n):
            p, hp, c = chunks[n]
            if p == 1 and hp == 0 and c == 0:
                # fc weights aren't needed until pair 2's fc round
                for et in range(8):
                    nc.sync.dma_start(wo_tiles[et][:],
                                      wo_t[et * 128:(et + 1) * 128, :])
                nc.sync.dma_start(id_t[:], ident)
            kt, kdA, kdB = st["k", p]
            qt, qdA, qdB = st["q", p]
            if kdA is None and hp == 0:
                # pair 0 head 0: no A-dups; both S matmuls in row group 0
                # (serial on PE - fine, startup is HAM-cold anyway)
                r0 = r1 = slice(0, 64)
                kt0 = kt1 = kt
                qt0 = qt1 = qt
            elif hp == 0:
                kt0, qt0, r0 = kt, qt, slice(0, 64)
                kt1, qt1, r1 = kdA, qdA, slice(64, 128)
            else:
                kt0, qt0, r0 = kdB, qdB, slice(0, 64)
                kt1, qt1, r1 = kt, qt, slice(64, 128)
            kb0, kb1 = 2 * c, 2 * c + 1
            ps = ps_s.tile([128, 1024], FP, tag="s", name=f"s{p}_{hp}_{c}")
            nc.tensor.matmul(
                ps[:, 0:512],
                lhsT=kt0[r0, kb0 * 128:(kb0 + 1) * 128],
                rhs=qt0[r0, :],
                start=True, stop=True)
            nc.tensor.matmul(
                ps[:, 512:1024],
                lhsT=kt1[r1, kb1 * 128:(kb1 + 1) * 128],
                rhs=qt1[r1, :],
                start=True, stop=True)
            # exp(s/8 - 4): the -4 shift cancels in softmax and keeps
            # max P ~= e^7 well inside fp16 range
            pt_ = ptp.tile([128, 1024], MD, name="pt_")
            nc.scalar.activation(pt_[:], ps[:], Exp,
                                 scale=0.125, bias=nbias[:])
            pts[n] = pt_

        def stage_pv(n):
            p, hp, c = chunks[n]
            if c == 0:
                pos[p, hp] = ps_o.tile([65, 512], FP, tag="o",
                                       name=f"po{p}_{hp}")
            po = pos[p, hp]
            v = st["v", p]
            pt_ = pts.pop(n)
            kb0, kb1 = 2 * c, 2 * c + 1
            nc.tensor.matmul(
                po[:],
                lhsT=v[:, kb0 * 130 + hp * 65:kb0 * 130 + hp * 65 + 65],
                rhs=pt_[:, 0:512],
                start=(c == 0), stop=False, skip_group_check=True)
            nc.tensor.matmul(
                po[:],
                lhsT=v[:, kb1 * 130 + hp * 65:kb1 * 130 + hp * 65 + 65],
                rhs=pt_[:, 512:1024],
                start=False, stop=(c == 7), skip_group_check=True)
            if c == 7:
                emit_norm(p, hp, pos.pop((p, hp)))

        def interleave(n):
            # next-pair projections and fc-round tiles, one small piece per
            # chunk slot so no emission point exceeds the per-chunk ACT
            # budget (~1.1us)
            p, hp, c = chunks[n]
            t = hp * 8 + c
            if t == 1 and p < 6:
                emit_loads(p + 2)
                return
            # pair 0 defers its projection slots past the HAM-cold window
            tt = t - 4 if p == 0 else t
            if tt == 2 and p < 7:
                emit_kproj(p + 1, 0)
            elif tt == 3 and p < 7:
                emit_kproj(p + 1, 1)
            elif tt == 4 and p < 7:
                emit_vproj(p + 1, 0)
            elif tt == 5 and p < 7:
                emit_vproj(p + 1, 1)
            elif tt == 6 and p < 7:
                emit_qproj(p + 1)
            elif 7 <= t <= 10 and p >= 2:
                # fc round for an earlier pair couple, 4 tiles on the even
                # pair and 4 on the odd pair, so no single pair carries the
                # whole 16-matmul round
                if p % 2 == 0:
                    emit_fc_tile(p - 2, p - 1, t - 7)
                else:
                    emit_fc_tile(p - 3, p - 2, t - 3)

        for n in range(NCH + 1):
            if n < NCH:
                stage_S_exp(n)
            if 0 <= n - 1 < NCH:
                stage_pv(n - 1)
            if n < NCH:
                interleave(n)

        for i in range(4):
            fc_final_early(i)
        for i in range(4):
            fc_final_late(i)
            fc_final_early(i + 4)
        for i in range(4, 8):
            fc_final_late(i)


# ---------------------------------------------------------------------------
# host side
# ---------------------------------------------------------------------------

_NC_CACHE = {}


def _get_nc():
    if KDT not in _NC_CACHE:
        _NC_CACHE[KDT] = build_nc(KDT)
    return _NC_CACHE[KDT]


def _np_dt():
    if KDT == "bf16":
        import ml_dtypes
        return ml_dtypes.bfloat16
    if KDT == "fp16":
        return np.float16
    return np.float32


def _bd(w):
    """128x128 block-diag of W.T (two copies)."""
    wt = np.ascontiguousarray(np.asarray(w).T.astype(np.float32))
    o = np.zeros((128, 128), np.float32)
    o[:64, :64] = wt
    o[64:, 64:] = wt
    return o


def kernel(values, keys, queries, Wv, Wk, Wq, Wo, bo):
    values = np.asarray(values, np.float32)
    keys = np.asarray(keys, np.float32)
    queries = np.asarray(queries, np.float32)

    dt = _np_dt()
    ident = np.eye(128, dtype=np.float32).astype(dt)
    wq_bd = _bd(Wq).astype(dt)
    wk_bd = _bd(Wk).astype(dt)
    wv_bd = _bd(Wv).astype(dt)
    wo_t = np.ascontiguousarray(np.asarray(Wo, np.float32).T).astype(dt)
    bo_r = np.ascontiguousarray(np.asarray(bo, np.float32).reshape(1, E))

    xk_t = [np.ascontiguousarray(keys[b].T).astype(dt) for b in range(B)]
    xv_t = [np.ascontiguousarray(values[b].T).astype(dt) for b in range(B)]

    in_maps = []
    for c in range(NCORES):
        b, qi = c // 4, c % 4
        in_maps.append({
            "xq_t": np.ascontiguousarray(
                queries[b, qi * SQ:(qi + 1) * SQ, :].T).astype(dt),
            "xk_t": xk_t[b],
            "xv_t": xv_t[b],
            "wq_bd": wq_bd, "wk_bd": wk_bd, "wv_bd": wv_bd,
            "wo_t": wo_t, "bo": bo_r, "ident": ident,
        })

    nc = _get_nc()
    res = run_bass_kernel_spmd(nc, in_maps, list(range(NCORES)),
                               trace=bool(int(os.environ.get("BASS_TRACE", "0"))))
    full = np.empty((B, S, E), np.float32)
    for c in range(NCORES):
        b, qi = c // 4, c % 4
        full[b, qi * SQ:(qi + 1) * SQ, :] = res.results[c]["out"]
    kernel.last_results = res
    return full
